# revision 13
# baseline (speedup 1.0000x reference)
"""BiConvLSTM kernel for one TRN2 chip (8 NeuronCores).

Strategy: 8-way model parallelism over the LSTM gate rows.
  - Each core holds a 1/8 slice of W_ih / W_hh rows (gate-aligned: for its
    288 hidden units it holds the i, f, g, o rows) resident in SBUF.
  - The input projection x @ W_ih.T (+ biases, via an augmented ones-row)
    is computed once for all (sample, t) pairs.
  - The 16 recurrence steps process all 16 sequences (8 samples x 2
    directions) together: gates = xproj[t] + h @ W_hh_shard.T, with h
    stationary (lhsT) and the weight shard as the moving operand (float32r
    -> full PE rate at N>=256).
  - Each core computes its 288-hidden-unit slice of h/c; the h slices are
    exchanged every step with an 8-rank AllGather (the only collective).
  - Epilogue (tanh + 1x1 conv over the 2 direction channels + leaky relu)
    runs on-device; the host concatenates the 8 hidden slices.

Perf notes (from neuron-profile iteration):
  - Weights are host-pre-permuted into the exact SBUF layout so each
    resident tensor loads with a single large DMA (DMA issue costs ~0.6us
    each on one queue).
  - DMA issue is spread across engine queues (sync/vector/scalar/gpsimd).
  - The 4 gate accumulation groups are ordered on the PE via explicit dep
    edges so gate i's PSUM completes early and its add/activation pipeline
    under the remaining matmuls.
  - Dummy f32 matmuls bridge the AllGather gap to keep the PE HAM
    clock-gate warm (idle >3.4us re-throttles the PE to 1.2 GHz).
  - A throwaway AllGather at kernel start absorbs the ~15us ncfw
    first-collective warmup while weights are still loading.

Sequence-row layout (row r of the 16-row state tiles):
  r 0..3   fwd direction, samples 0,2,4,6
  r 4..7   bwd direction, samples 0,2,4,6
  r 8..11  fwd direction, samples 1,3,5,7
  r 12..15 bwd direction, samples 1,3,5,7
This ordering makes the reference's stack([h_fwd, h_bwd]).reshape(B, 2, ...)
channel pairing come out as (row b', row b'+8) for output batch b'.
"""

import sys

if "/opt/trn_rl_repo" not in sys.path:
    sys.path.append("/opt/trn_rl_repo")

import ml_dtypes
import numpy as np

from concourse import bacc, bass_utils, mybir, tile
from concourse.tile import add_dep_helper

B, T, H, W = 8, 16, 48, 48
HW = H * W              # 2304
NSEQ = 16               # 8 samples x 2 directions
NC = 8                  # cores
S = HW // NC            # 288 hidden units per core
G4 = 4 * S              # 1152 gate rows per core
KT = HW // 128          # 18 K-tiles over the hidden dim
KTA = KT + 1            # +1 tile holding the bias ones-row
NCHUNK = 4              # wih stream chunks (5 k-tiles each, last padded)
KCH = 5
NDUM = 12               # HAM keep-warm dummy matmuls per step
F32 = mybir.dt.float32
F32R = mybir.dt.float32r
BF16 = mybir.dt.bfloat16

# samples laid out as columns j=0..7 per timestep: j<4 -> sample 2j, else 2(j-4)+1
SAMP_ORDER = [0, 2, 4, 6, 1, 3, 5, 7]


def _build(w0: float, w1: float, cb: float):
    nc = bacc.Bacc("TRN2", target_bir_lowering=False, debug=False, num_devices=NC)

    xT_d = nc.dram_tensor("xT", [128, KTA, 128], BF16, kind="ExternalInput")
    wih_d = nc.dram_tensor("wih", [NCHUNK, 128, KCH, G4], BF16, kind="ExternalInput")
    whh_d = nc.dram_tensor("whh", [128, KT, G4], BF16, kind="ExternalInput")
    eye_d = nc.dram_tensor("eye16", [16, 16], F32, kind="ExternalInput")
    out_d = nc.dram_tensor("out", [B, S], F32, kind="ExternalOutput")

    SIG = mybir.ActivationFunctionType.Sigmoid
    TANH = mybir.ActivationFunctionType.Tanh
    ADD = mybir.AluOpType.add
    MULT = mybir.AluOpType.mult
    MAX = mybir.AluOpType.max

    with tile.TileContext(nc) as tc:
        with (
            tc.tile_pool(name="const", bufs=1) as constp,
            tc.tile_pool(name="wstream", bufs=2) as wsp,
            tc.tile_pool(name="stage", bufs=2) as stp,
            tc.tile_pool(name="ew", bufs=2) as ewp,
            tc.tile_pool(name="state", bufs=2) as statep,
            tc.tile_pool(name="pg", bufs=4, space="PSUM") as pgp,
            tc.tile_pool(name="pt", bufs=2, space="PSUM") as ptp,
            tc.tile_pool(name="pd", bufs=1, space="PSUM") as pdp,
            tc.tile_pool(name="dram", bufs=2, space="DRAM") as dp,
        ):
            # throwaway collective: pays the ncfw first-call cost while the
            # weight DMAs stream in
            warm_in = dp.tile([S, NSEQ], BF16, tag="warmin")
            warm_out = dp.tile([128, KT, NSEQ], BF16, addr_space="Shared", tag="warmout")
            warm_out2 = dp.tile(
                [128, KT, NSEQ], BF16, addr_space="Shared", tag="warmout2"
            )
            for wout in (warm_out, warm_out2):
                nc.gpsimd.collective_compute(
                    "AllGather",
                    mybir.AluOpType.bypass,
                    ins=[warm_in.opt()],
                    outs=[wout.opt()],
                    replica_groups=[list(range(NC))],
                )

            eye_sb = constp.tile([16, 16], F32, tag="eye")
            nc.scalar.dma_start(eye_sb[:, :], eye_d[:, :])
            # touch sigmoid/tanh early so the ACT table set loads during the
            # weight DMAs instead of on step 0's critical path
            actwarm = constp.tile([16, 16], F32, tag="actwarm")
            nc.scalar.activation(
                actwarm[:, :], eye_sb[:, :], mybir.ActivationFunctionType.Sigmoid
            )

            xT_sb = constp.tile([128, KTA, 128], BF16, tag="xT")
            nc.scalar.dma_start(xT_sb[:, :, :], xT_d[:, :, :])

            # one dma_start lands on one queue (~130 GB/s); split the big
            # resident loads across queues/engines for bandwidth
            whh_sb = constp.tile([128, KT, G4], BF16, tag="whh")
            whh_engines = [nc.gpsimd, nc.sync, nc.scalar]
            for piece in range(6):
                k0 = piece * 3
                whh_engines[piece % 3].dma_start(
                    whh_sb[:, k0 : k0 + 3, :], whh_d[:, k0 : k0 + 3, :]
                )

            # hidden state, transposed layout: [128, k-tile, seq]
            hT_sb = constp.tile([128, KT, NSEQ], BF16, tag="hT")

            # ---- xproj = [xT; ones].T @ [W_ih_shard.T; bias] for all 128 cols
            px = [pgp.tile([128, S], F32, tag="g", name=f"px{gi}") for gi in range(4)]
            for c in range(NCHUNK):
                wkc = wsp.tile([128, KCH, G4], BF16, tag="wk")
                [nc.sync, nc.scalar, nc.gpsimd, nc.sync][c].dma_start(
                    wkc[:, :, :], wih_d[c]
                )
                for kk in range(KCH):
                    k = c * KCH + kk
                    if k >= KTA:
                        break
                    for gi in range(4):
                        nc.tensor.matmul(
                            px[gi][:, :],
                            lhsT=xT_sb[:, k, :],
                            rhs=wkc[:, kk, gi * S : (gi + 1) * S],
                            start=(k == 0),
                            stop=(k == KTA - 1),
                        )
            xproj_sb = constp.tile([128, G4], F32, tag="xproj")
            for gi in range(4):
                nc.vector.tensor_copy(xproj_sb[:, gi * S : (gi + 1) * S], px[gi][:, :])

            c_prev = None
            h_last = None
            for s in range(T):
                # xproj rows for this step: fwd uses t=s, bwd uses t=15-s
                stage = stp.tile([NSEQ, G4], F32, tag="stage")
                nc.sync.dma_start(stage[0:4, :], xproj_sb[8 * s : 8 * s + 4, :])
                nc.sync.dma_start(stage[8:12, :], xproj_sb[8 * s + 4 : 8 * s + 8, :])
                nc.scalar.dma_start(stage[4:8, :], xproj_sb[120 - 8 * s : 124 - 8 * s, :])
                nc.scalar.dma_start(
                    stage[12:16, :], xproj_sb[124 - 8 * s : 128 - 8 * s, :]
                )

                acts = []
                prev_gate_last_mm = None
                for gi in range(4):
                    if s == 0:
                        src = stage[:, gi * S : (gi + 1) * S]
                    else:
                        pg = pgp.tile([NSEQ, S], F32, tag="g")
                        first_mm = None
                        last_mm = None
                        for k in range(KT):
                            mm = nc.tensor.matmul(
                                pg[:, :],
                                lhsT=hT_sb[:, k, :],
                                rhs=whh_sb[:, k, gi * S : (gi + 1) * S],
                                start=(k == 0),
                                stop=(k == KT - 1),
                            )
                            if first_mm is None:
                                first_mm = mm
                            last_mm = mm
                        # keep the 4 accumulation groups contiguous on the PE so
                        # gate gi's PSUM is complete early and its epilogue
                        # overlaps the remaining gates' matmuls
                        if prev_gate_last_mm is not None:
                            add_dep_helper(
                                first_mm.ins,
                                prev_gate_last_mm.ins,
                                False,
                                reason="gate group order",
                            )
                        prev_gate_last_mm = last_mm
                        gsum = ewp.tile([NSEQ, S], F32, tag="gsum")
                        nc.vector.tensor_tensor(
                            gsum[:, :], pg[:, :], stage[:, gi * S : (gi + 1) * S], ADD
                        )
                        src = gsum[:, :]
                    act = ewp.tile([NSEQ, S], F32, tag=f"act{gi}")
                    nc.scalar.activation(act[:, :], src, TANH if gi == 2 else SIG)
                    acts.append(act)
                i_, f_, g_, o_ = acts

                m1 = ewp.tile([NSEQ, S], F32, tag="m1")
                nc.vector.tensor_tensor(m1[:, :], i_[:, :], g_[:, :], MULT)
                c_new = statep.tile([NSEQ, S], F32, tag="c")
                if s == 0:
                    nc.vector.tensor_copy(c_new[:, :], m1[:, :])
                else:
                    fc = ewp.tile([NSEQ, S], F32, tag="fc")
                    nc.vector.tensor_tensor(fc[:, :], f_[:, :], c_prev[:, :], MULT)
                    nc.vector.tensor_tensor(c_new[:, :], fc[:, :], m1[:, :], ADD)
                c_prev = c_new

                tanh_c = ewp.tile([NSEQ, S], F32, tag="tanh_c")
                nc.scalar.activation(tanh_c[:, :], c_new[:, :], TANH)
                h_new = statep.tile([NSEQ, S], F32, tag="h")

                if s < T - 1:
                    nc.vector.tensor_tensor(h_new[:, :], o_[:, :], tanh_c[:, :], MULT)
                    # transpose own h slice to [hid, seq] and all-gather
                    hts = ewp.tile([96, 3, NSEQ], BF16, tag="hts")
                    cc_in = dp.tile([S, NSEQ], BF16, tag="ccin")
                    ccin_engines = [nc.sync, nc.scalar, nc.gpsimd]
                    for j in range(3):
                        tp = ptp.tile([96, NSEQ], F32, tag="tp")
                        last_tp = nc.tensor.transpose(
                            tp[:, :], h_new[:, 96 * j : 96 * (j + 1)], eye_sb[:, :]
                        )
                        nc.vector.tensor_copy(hts[:, j, :], tp[:, :])
                        ccin_engines[j].dma_start(
                            cc_in[96 * j : 96 * (j + 1), :], hts[:, j, :]
                        )
                    # with partition-residue hidden sharding, the rank-major
                    # AllGather output IS the [128, k, seq] hT layout
                    cc_out = dp.tile(
                        [128, KT, NSEQ], BF16, addr_space="Shared", tag="ccout"
                    )
                    nc.gpsimd.collective_compute(
                        "AllGather",
                        mybir.AluOpType.bypass,
                        ins=[cc_in.opt()],
                        outs=[cc_out.opt()],
                        replica_groups=[list(range(NC))],
                    )
                    nc.sync.dma_start(hT_sb[:, 0:9, :], cc_out[:, 0:9, :])
                    nc.scalar.dma_start(hT_sb[:, 9:KT, :], cc_out[:, 9:KT, :])
                    # dummy f32 matmuls chained behind the transposes bridge the
                    # AllGather gap so the PE HAM clock-gate stays warm
                    dum = pdp.tile([NSEQ, 512], F32, tag="dum")
                    first_dum = None
                    for _ in range(NDUM):
                        dmm = nc.tensor.matmul(
                            dum[:, :],
                            lhsT=xproj_sb[:, 0:16],
                            rhs=xproj_sb[:, 0:512],
                            start=True,
                            stop=True,
                        )
                        if first_dum is None:
                            first_dum = dmm
                    add_dep_helper(
                        first_dum.ins, last_tp.ins, False, reason="dummies after transpose"
                    )
                else:
                    nc.vector.tensor_tensor(h_new[:, :], o_[:, :], tanh_c[:, :], MULT)
                    h_last = h_new  # noqa

            # ---- epilogue: y[b'] = leaky(w0*tanh(h[b']) + w1*tanh(h[b'+8]) + cb)
            th = ewp.tile([NSEQ, S], F32, tag="th")
            nc.scalar.activation(th[:, :], h_last[:, :], TANH)
            thb = ewp.tile([B, S], F32, tag="thb")
            nc.sync.dma_start(thb[:, :], th[8:16, :])
            ya = ewp.tile([B, S], F32, tag="ya")
            nc.vector.tensor_scalar_mul(ya[:, :], th[0:8, :], w0)
            yb = ewp.tile([B, S], F32, tag="yb")
            nc.vector.tensor_scalar(yb[:, :], thb[:, :], w1, cb, MULT, ADD)
            yc = ewp.tile([B, S], F32, tag="yc")
            nc.vector.tensor_tensor(yc[:, :], ya[:, :], yb[:, :], ADD)
            yd = ewp.tile([B, S], F32, tag="yd")
            nc.vector.tensor_scalar_mul(yd[:, :], yc[:, :], 0.01)
            ye = ewp.tile([B, S], F32, tag="ye")
            nc.vector.tensor_tensor(ye[:, :], yc[:, :], yd[:, :], MAX)
            nc.sync.dma_start(out_d[:, :], ye[:, :])

    nc.compile()
    return nc


def _prep_inputs(x, W_ih, W_hh, b_ih, b_hh):
    """Build the 8 per-core input maps (SBUF-layout pre-permuted)."""
    xr = np.ascontiguousarray(x, dtype=np.float32).reshape(B, T, HW)
    # columns n = t*8 + j, sample order per t given by SAMP_ORDER
    Xc = xr[SAMP_ORDER].transpose(1, 0, 2).reshape(B * T, HW)  # [128, 2304]
    xT = np.zeros((KTA * 128, 128), dtype=np.float32)
    xT[:HW] = Xc.T
    xT[HW] = 1.0
    # -> [128, KTA, 128] partition-major
    xT = np.ascontiguousarray(
        xT.reshape(KTA, 128, 128).transpose(1, 0, 2)
    ).astype(ml_dtypes.bfloat16)

    bias = (b_ih + b_hh).astype(np.float32)
    eye = np.eye(16, dtype=np.float32)

    in_maps = []
    for core in range(NC):
        # partition-residue sharding: core r owns units u with
        # u%128 in [16r, 16r+16); local index j = p'*KT + k <-> u = 128k+16r+p'
        j = np.arange(S)
        hid = 128 * (j % KT) + 16 * core + j // KT
        rows = np.concatenate([gi * HW + hid for gi in range(4)])  # i|f|g|o
        wih = np.zeros((NCHUNK * KCH * 128, G4), dtype=np.float32)
        wih[:HW] = W_ih[rows].T
        wih[HW] = bias[rows]
        # -> [NCHUNK, 128, KCH, G4]
        wih = np.ascontiguousarray(
            wih.reshape(NCHUNK, KCH, 128, G4).transpose(0, 2, 1, 3)
        ).astype(ml_dtypes.bfloat16)
        whh = W_hh[rows].T.reshape(KT, 128, G4).transpose(1, 0, 2)
        in_maps.append(
            {
                "xT": xT,
                "wih": wih,
                "whh": np.ascontiguousarray(whh).astype(ml_dtypes.bfloat16),
                "eye16": eye,
            }
        )
    return in_maps


def run(x, W_ih, W_hh, b_ih, b_hh, conv_w, conv_b, trace=False, tmpdir=None):
    """Build + run on 8 cores; returns (full_output, BassKernelResults)."""
    w0 = float(np.asarray(conv_w).reshape(2)[0])
    w1 = float(np.asarray(conv_w).reshape(2)[1])
    cb = float(np.asarray(conv_b).reshape(1)[0])
    nc = _build(w0, w1, cb)
    in_maps = _prep_inputs(
        np.asarray(x), np.asarray(W_ih), np.asarray(W_hh),
        np.asarray(b_ih), np.asarray(b_hh),
    )
    res = bass_utils.run_bass_kernel_spmd(
        nc, in_maps, core_ids=list(range(NC)), trace=trace, tmpdir=tmpdir
    )
    y = np.empty((B, HW), dtype=np.float32)
    j = np.arange(S)
    for core in range(NC):
        u = 128 * (j % KT) + 16 * core + j // KT
        y[:, u] = res.results[core]["out"]
    return y.reshape(B, 1, H, W).astype(np.float32), res


def kernel(x, W_ih, W_hh, b_ih, b_hh, conv_w, conv_b):
    y, _ = run(x, W_ih, W_hh, b_ih, b_hh, conv_w, conv_b, trace=False)
    return y


# revision 14
# speedup vs baseline: 1.0015x; 1.0015x over previous
"""BiConvLSTM kernel for one TRN2 chip (8 NeuronCores).

Strategy: 8-way model parallelism over the LSTM gate rows.
  - Each core holds a 1/8 slice of W_ih / W_hh rows (gate-aligned: for its
    288 hidden units it holds the i, f, g, o rows) resident in SBUF.
  - The input projection x @ W_ih.T (+ biases, via an augmented ones-row)
    is computed once for all (sample, t) pairs.
  - The 16 recurrence steps process all 16 sequences (8 samples x 2
    directions) together: gates = xproj[t] + h @ W_hh_shard.T, with h
    stationary (lhsT) and the weight shard as the moving operand (float32r
    -> full PE rate at N>=256).
  - Each core computes its 288-hidden-unit slice of h/c; the h slices are
    exchanged every step with an 8-rank AllGather (the only collective).
  - Epilogue (tanh + 1x1 conv over the 2 direction channels + leaky relu)
    runs on-device; the host concatenates the 8 hidden slices.

Perf notes (from neuron-profile iteration):
  - Weights are host-pre-permuted into the exact SBUF layout so each
    resident tensor loads with a single large DMA (DMA issue costs ~0.6us
    each on one queue).
  - DMA issue is spread across engine queues (sync/vector/scalar/gpsimd).
  - The 4 gate accumulation groups are ordered on the PE via explicit dep
    edges so gate i's PSUM completes early and its add/activation pipeline
    under the remaining matmuls.
  - Dummy f32 matmuls bridge the AllGather gap to keep the PE HAM
    clock-gate warm (idle >3.4us re-throttles the PE to 1.2 GHz).
  - A throwaway AllGather at kernel start absorbs the ~15us ncfw
    first-collective warmup while weights are still loading.

Sequence-row layout (row r of the 16-row state tiles):
  r 0..3   fwd direction, samples 0,2,4,6
  r 4..7   bwd direction, samples 0,2,4,6
  r 8..11  fwd direction, samples 1,3,5,7
  r 12..15 bwd direction, samples 1,3,5,7
This ordering makes the reference's stack([h_fwd, h_bwd]).reshape(B, 2, ...)
channel pairing come out as (row b', row b'+8) for output batch b'.
"""

import sys

if "/opt/trn_rl_repo" not in sys.path:
    sys.path.append("/opt/trn_rl_repo")

import ml_dtypes
import numpy as np

from concourse import bacc, bass_utils, mybir, tile
from concourse.tile import add_dep_helper

B, T, H, W = 8, 16, 48, 48
HW = H * W              # 2304
NSEQ = 16               # 8 samples x 2 directions
NC = 8                  # cores
S = HW // NC            # 288 hidden units per core
G4 = 4 * S              # 1152 gate rows per core
KT = HW // 128          # 18 K-tiles over the hidden dim
KTA = KT + 1            # +1 tile holding the bias ones-row
NCHUNK = 4              # wih stream chunks (5 k-tiles each, last padded)
KCH = 5
NDUM = 12               # HAM keep-warm dummy matmuls per step
F32 = mybir.dt.float32
F32R = mybir.dt.float32r
BF16 = mybir.dt.bfloat16

# samples laid out as columns j=0..7 per timestep: j<4 -> sample 2j, else 2(j-4)+1
SAMP_ORDER = [0, 2, 4, 6, 1, 3, 5, 7]


def _build(w0: float, w1: float, cb: float):
    nc = bacc.Bacc("TRN2", target_bir_lowering=False, debug=False, num_devices=NC)

    xT_d = nc.dram_tensor("xT", [128, KTA, 128], BF16, kind="ExternalInput")
    wih_d = nc.dram_tensor("wih", [NCHUNK, 128, KCH, G4], BF16, kind="ExternalInput")
    whh_d = nc.dram_tensor("whh", [128, KT, G4], BF16, kind="ExternalInput")
    eye_d = nc.dram_tensor("eye16", [16, 16], F32, kind="ExternalInput")
    out_d = nc.dram_tensor("out", [B, S], F32, kind="ExternalOutput")

    SIG = mybir.ActivationFunctionType.Sigmoid
    TANH = mybir.ActivationFunctionType.Tanh
    ADD = mybir.AluOpType.add
    MULT = mybir.AluOpType.mult
    MAX = mybir.AluOpType.max

    with tile.TileContext(nc) as tc:
        with (
            tc.tile_pool(name="const", bufs=1) as constp,
            tc.tile_pool(name="wstream", bufs=2) as wsp,
            tc.tile_pool(name="stage", bufs=2) as stp,
            tc.tile_pool(name="ew", bufs=2) as ewp,
            tc.tile_pool(name="state", bufs=2) as statep,
            tc.tile_pool(name="pg", bufs=4, space="PSUM") as pgp,
            tc.tile_pool(name="pt", bufs=2, space="PSUM") as ptp,
            tc.tile_pool(name="pd", bufs=1, space="PSUM") as pdp,
            tc.tile_pool(name="dram", bufs=2, space="DRAM") as dp,
        ):
            # throwaway collective: pays the ncfw first-call cost while the
            # weight DMAs stream in
            warm_in = dp.tile([S, NSEQ], BF16, tag="warmin")
            warm_out = dp.tile([128, KT, NSEQ], BF16, addr_space="Shared", tag="warmout")
            nc.gpsimd.collective_compute(
                "AllGather",
                mybir.AluOpType.bypass,
                ins=[warm_in.opt()],
                outs=[warm_out.opt()],
                replica_groups=[list(range(NC))],
            )

            eye_sb = constp.tile([16, 16], F32, tag="eye")
            nc.scalar.dma_start(eye_sb[:, :], eye_d[:, :])
            # touch sigmoid/tanh early so the ACT table set loads during the
            # weight DMAs instead of on step 0's critical path
            actwarm = constp.tile([16, 16], F32, tag="actwarm")
            nc.scalar.activation(
                actwarm[:, :], eye_sb[:, :], mybir.ActivationFunctionType.Sigmoid
            )

            xT_sb = constp.tile([128, KTA, 128], BF16, tag="xT")
            nc.scalar.dma_start(xT_sb[:, :, :], xT_d[:, :, :])

            # one dma_start lands on one queue (~130 GB/s); split the big
            # resident loads across queues/engines for bandwidth
            whh_sb = constp.tile([128, KT, G4], BF16, tag="whh")
            whh_engines = [nc.gpsimd, nc.sync, nc.scalar]
            for piece in range(6):
                k0 = piece * 3
                whh_engines[piece % 3].dma_start(
                    whh_sb[:, k0 : k0 + 3, :], whh_d[:, k0 : k0 + 3, :]
                )

            # hidden state, transposed layout: [128, k-tile, seq]
            hT_sb = constp.tile([128, KT, NSEQ], BF16, tag="hT")

            # ---- xproj = [xT; ones].T @ [W_ih_shard.T; bias] for all 128 cols
            px = [pgp.tile([128, S], F32, tag="g", name=f"px{gi}") for gi in range(4)]
            for c in range(NCHUNK):
                wkc = wsp.tile([128, KCH, G4], BF16, tag="wk")
                [nc.sync, nc.scalar, nc.gpsimd, nc.sync][c].dma_start(
                    wkc[:, :, :], wih_d[c]
                )
                for kk in range(KCH):
                    k = c * KCH + kk
                    if k >= KTA:
                        break
                    for gi in range(4):
                        nc.tensor.matmul(
                            px[gi][:, :],
                            lhsT=xT_sb[:, k, :],
                            rhs=wkc[:, kk, gi * S : (gi + 1) * S],
                            start=(k == 0),
                            stop=(k == KTA - 1),
                        )
            xproj_sb = constp.tile([128, G4], F32, tag="xproj")
            for gi in range(4):
                nc.vector.tensor_copy(xproj_sb[:, gi * S : (gi + 1) * S], px[gi][:, :])

            c_prev = None
            h_last = None
            for s in range(T):
                # xproj rows for this step: fwd uses t=s, bwd uses t=15-s
                stage = stp.tile([NSEQ, G4], F32, tag="stage")
                nc.sync.dma_start(stage[0:4, :], xproj_sb[8 * s : 8 * s + 4, :])
                nc.sync.dma_start(stage[8:12, :], xproj_sb[8 * s + 4 : 8 * s + 8, :])
                nc.scalar.dma_start(stage[4:8, :], xproj_sb[120 - 8 * s : 124 - 8 * s, :])
                nc.scalar.dma_start(
                    stage[12:16, :], xproj_sb[124 - 8 * s : 128 - 8 * s, :]
                )

                acts = []
                prev_gate_last_mm = None
                for gi in range(4):
                    if s == 0:
                        src = stage[:, gi * S : (gi + 1) * S]
                    else:
                        pg = pgp.tile([NSEQ, S], F32, tag="g")
                        first_mm = None
                        last_mm = None
                        for k in range(KT):
                            mm = nc.tensor.matmul(
                                pg[:, :],
                                lhsT=hT_sb[:, k, :],
                                rhs=whh_sb[:, k, gi * S : (gi + 1) * S],
                                start=(k == 0),
                                stop=(k == KT - 1),
                            )
                            if first_mm is None:
                                first_mm = mm
                            last_mm = mm
                        # keep the 4 accumulation groups contiguous on the PE so
                        # gate gi's PSUM is complete early and its epilogue
                        # overlaps the remaining gates' matmuls
                        if prev_gate_last_mm is not None:
                            add_dep_helper(
                                first_mm.ins,
                                prev_gate_last_mm.ins,
                                False,
                                reason="gate group order",
                            )
                        prev_gate_last_mm = last_mm
                        gsum = ewp.tile([NSEQ, S], F32, tag="gsum")
                        nc.vector.tensor_tensor(
                            gsum[:, :], pg[:, :], stage[:, gi * S : (gi + 1) * S], ADD
                        )
                        src = gsum[:, :]
                    act = ewp.tile([NSEQ, S], F32, tag=f"act{gi}")
                    nc.scalar.activation(act[:, :], src, TANH if gi == 2 else SIG)
                    acts.append(act)
                i_, f_, g_, o_ = acts

                m1 = ewp.tile([NSEQ, S], F32, tag="m1")
                nc.vector.tensor_tensor(m1[:, :], i_[:, :], g_[:, :], MULT)
                c_new = statep.tile([NSEQ, S], F32, tag="c")
                if s == 0:
                    nc.vector.tensor_copy(c_new[:, :], m1[:, :])
                else:
                    fc = ewp.tile([NSEQ, S], F32, tag="fc")
                    nc.vector.tensor_tensor(fc[:, :], f_[:, :], c_prev[:, :], MULT)
                    nc.vector.tensor_tensor(c_new[:, :], fc[:, :], m1[:, :], ADD)
                c_prev = c_new

                tanh_c = ewp.tile([NSEQ, S], F32, tag="tanh_c")
                nc.scalar.activation(tanh_c[:, :], c_new[:, :], TANH)
                h_new = statep.tile([NSEQ, S], F32, tag="h")

                if s < T - 1:
                    nc.vector.tensor_tensor(h_new[:, :], o_[:, :], tanh_c[:, :], MULT)
                    # transpose own h slice to [hid, seq] and all-gather
                    hts = ewp.tile([96, 3, NSEQ], BF16, tag="hts")
                    cc_in = dp.tile([S, NSEQ], BF16, tag="ccin")
                    ccin_engines = [nc.sync, nc.scalar, nc.gpsimd]
                    for j in range(3):
                        tp = ptp.tile([96, NSEQ], F32, tag="tp")
                        last_tp = nc.tensor.transpose(
                            tp[:, :], h_new[:, 96 * j : 96 * (j + 1)], eye_sb[:, :]
                        )
                        nc.vector.tensor_copy(hts[:, j, :], tp[:, :])
                        ccin_engines[j].dma_start(
                            cc_in[96 * j : 96 * (j + 1), :], hts[:, j, :]
                        )
                    # with partition-residue hidden sharding, the rank-major
                    # AllGather output IS the [128, k, seq] hT layout
                    cc_out = dp.tile(
                        [128, KT, NSEQ], BF16, addr_space="Shared", tag="ccout"
                    )
                    nc.gpsimd.collective_compute(
                        "AllGather",
                        mybir.AluOpType.bypass,
                        ins=[cc_in.opt()],
                        outs=[cc_out.opt()],
                        replica_groups=[list(range(NC))],
                    )
                    nc.sync.dma_start(hT_sb[:, :, :], cc_out[:, :, :])
                    # dummy f32 matmuls chained behind the transposes bridge the
                    # AllGather gap so the PE HAM clock-gate stays warm
                    dum = pdp.tile([NSEQ, 512], F32, tag="dum")
                    first_dum = None
                    for _ in range(NDUM):
                        dmm = nc.tensor.matmul(
                            dum[:, :],
                            lhsT=xproj_sb[:, 0:16],
                            rhs=xproj_sb[:, 0:512],
                            start=True,
                            stop=True,
                        )
                        if first_dum is None:
                            first_dum = dmm
                    add_dep_helper(
                        first_dum.ins, last_tp.ins, False, reason="dummies after transpose"
                    )
                else:
                    nc.vector.tensor_tensor(h_new[:, :], o_[:, :], tanh_c[:, :], MULT)
                    h_last = h_new  # noqa

            # ---- epilogue: y[b'] = leaky(w0*tanh(h[b']) + w1*tanh(h[b'+8]) + cb)
            th = ewp.tile([NSEQ, S], F32, tag="th")
            nc.scalar.activation(th[:, :], h_last[:, :], TANH)
            thb = ewp.tile([B, S], F32, tag="thb")
            nc.sync.dma_start(thb[:, :], th[8:16, :])
            ya = ewp.tile([B, S], F32, tag="ya")
            nc.vector.tensor_scalar_mul(ya[:, :], th[0:8, :], w0)
            yb = ewp.tile([B, S], F32, tag="yb")
            nc.vector.tensor_scalar(yb[:, :], thb[:, :], w1, cb, MULT, ADD)
            yc = ewp.tile([B, S], F32, tag="yc")
            nc.vector.tensor_tensor(yc[:, :], ya[:, :], yb[:, :], ADD)
            yd = ewp.tile([B, S], F32, tag="yd")
            nc.vector.tensor_scalar_mul(yd[:, :], yc[:, :], 0.01)
            ye = ewp.tile([B, S], F32, tag="ye")
            nc.vector.tensor_tensor(ye[:, :], yc[:, :], yd[:, :], MAX)
            nc.sync.dma_start(out_d[:, :], ye[:, :])

    nc.compile()
    return nc


def _prep_inputs(x, W_ih, W_hh, b_ih, b_hh):
    """Build the 8 per-core input maps (SBUF-layout pre-permuted)."""
    xr = np.ascontiguousarray(x, dtype=np.float32).reshape(B, T, HW)
    # columns n = t*8 + j, sample order per t given by SAMP_ORDER
    Xc = xr[SAMP_ORDER].transpose(1, 0, 2).reshape(B * T, HW)  # [128, 2304]
    xT = np.zeros((KTA * 128, 128), dtype=np.float32)
    xT[:HW] = Xc.T
    xT[HW] = 1.0
    # -> [128, KTA, 128] partition-major
    xT = np.ascontiguousarray(
        xT.reshape(KTA, 128, 128).transpose(1, 0, 2)
    ).astype(ml_dtypes.bfloat16)

    bias = (b_ih + b_hh).astype(np.float32)
    eye = np.eye(16, dtype=np.float32)

    in_maps = []
    for core in range(NC):
        # partition-residue sharding: core r owns units u with
        # u%128 in [16r, 16r+16); local index j = p'*KT + k <-> u = 128k+16r+p'
        j = np.arange(S)
        hid = 128 * (j % KT) + 16 * core + j // KT
        rows = np.concatenate([gi * HW + hid for gi in range(4)])  # i|f|g|o
        wih = np.zeros((NCHUNK * KCH * 128, G4), dtype=np.float32)
        wih[:HW] = W_ih[rows].T
        wih[HW] = bias[rows]
        # -> [NCHUNK, 128, KCH, G4]
        wih = np.ascontiguousarray(
            wih.reshape(NCHUNK, KCH, 128, G4).transpose(0, 2, 1, 3)
        ).astype(ml_dtypes.bfloat16)
        whh = W_hh[rows].T.reshape(KT, 128, G4).transpose(1, 0, 2)
        in_maps.append(
            {
                "xT": xT,
                "wih": wih,
                "whh": np.ascontiguousarray(whh).astype(ml_dtypes.bfloat16),
                "eye16": eye,
            }
        )
    return in_maps


def run(x, W_ih, W_hh, b_ih, b_hh, conv_w, conv_b, trace=False, tmpdir=None):
    """Build + run on 8 cores; returns (full_output, BassKernelResults)."""
    w0 = float(np.asarray(conv_w).reshape(2)[0])
    w1 = float(np.asarray(conv_w).reshape(2)[1])
    cb = float(np.asarray(conv_b).reshape(1)[0])
    nc = _build(w0, w1, cb)
    in_maps = _prep_inputs(
        np.asarray(x), np.asarray(W_ih), np.asarray(W_hh),
        np.asarray(b_ih), np.asarray(b_hh),
    )
    res = bass_utils.run_bass_kernel_spmd(
        nc, in_maps, core_ids=list(range(NC)), trace=trace, tmpdir=tmpdir
    )
    y = np.empty((B, HW), dtype=np.float32)
    j = np.arange(S)
    for core in range(NC):
        u = 128 * (j % KT) + 16 * core + j // KT
        y[:, u] = res.results[core]["out"]
    return y.reshape(B, 1, H, W).astype(np.float32), res


def kernel(x, W_ih, W_hh, b_ih, b_hh, conv_w, conv_b):
    y, _ = run(x, W_ih, W_hh, b_ih, b_hh, conv_w, conv_b, trace=False)
    return y
